# revision 1
# baseline (speedup 1.0000x reference)
"""Trainium2 Bass kernel for BERTForContrastiveLearningForTokenMetric loss.

Math: the reference loss factors into masked per-token sums:
    proto = (sum_{ent} x_t) / n_ent
    loss  = (sum_{nz} x_t/||x_t||) . proto / (||proto|| * n_tok)
so one pass over logits per core suffices.  Each core processes 8 of the 64
batches (4096 tokens), producing a [2, 768] partial:
    row 0 = sum_{ent tokens} x_t
    row 1 = sum_{nz tokens}  x_t / ||x_t||
The host sums partials across the 8 cores and does the tiny final combine.

Per-core device pipeline, per 512-token block (tokens laid out 4/partition):
    DMA 1.5 MiB x-block -> SBUF [128, 4, 768]
    DVE tensor_tensor_reduce (x*x, sum) -> sq [128, 4]     (per-token norms^2)
    DVE reciprocal -> 1/sq; ACT sqrt -> 1/||x||
    DVE mult (in place): aux nz slot <- nz / ||x||         (matmul weights)
    PE  matmul lhsT=[128,2] (ent, nz/||x||), rhs=x -> PSUM [2,768] accumulate
"""

import numpy as np

B, S, D = 64, 512, 768
N_CORES = 8
B_PER_CORE = B // N_CORES            # 8
TOK_PER_CORE = B_PER_CORE * S        # 4096
P = 128                              # SBUF partitions
J = 4                                # tokens per partition per block
BLK_TOK = P * J                      # 512 tokens per block
N_BLK = TOK_PER_CORE // BLK_TOK      # 8

_CACHE = {}


def _tile_program(nc, x_h, aux_h, out_h, repeat=1):
    """Emit the per-core Tile program.

    x_h   [N_BLK, P, J, D] f32 : logits shard, token t = i*512 + p*4 + j
    aux_h [P, N_BLK, J, 2] f32 : (ent_mask, nz_mask) per token
    out_h [2, D] f32           : partials (sum_ent x, sum_nz x/||x||)
    repeat: wrap the block loop in a dynamic For_i (timing harness only)
    """
    import concourse.tile as tile
    from concourse import mybir
    from contextlib import nullcontext

    f32 = mybir.dt.float32
    bf16 = mybir.dt.bfloat16
    OP = mybir.AluOpType
    AF = mybir.ActivationFunctionType

    with tile.TileContext(nc) as tc:
        with (
            tc.tile_pool(name="xp", bufs=5) as xp,
            tc.tile_pool(name="xbp", bufs=5) as xbp,
            tc.tile_pool(name="dump", bufs=3) as dumpp,
            tc.tile_pool(name="small", bufs=6) as small,
            tc.tile_pool(name="wp", bufs=4) as wp,
            tc.tile_pool(name="single", bufs=1) as single,
            tc.tile_pool(name="psum", bufs=1, space="PSUM") as psp,
        ):
            aux_sb = single.tile([P, N_BLK, J, 2], f32)
            nc.sync.dma_start(out=aux_sb[:], in_=aux_h[:])

            p512 = psp.tile([2, 512], f32)
            p256 = psp.tile([2, 256], f32)

            loop = tc.For_i(0, repeat, 1) if repeat > 1 else nullcontext()
            with loop:
                for i in range(N_BLK):
                    xb = xbp.tile([P, J, D], bf16)
                    # per-j-slice casting DMAs (gpsimd SWDGE converts
                    # fp32->bf16 in flight): fp32 matmuls stream 4 passes on
                    # the PE (4x cycles); bf16 operands restore 1 col/cycle
                    for jj in range(J):
                        nc.gpsimd.dma_start(out=xb[:, jj, :], in_=x_h[i, :, jj, :])

                    dump = dumpp.tile([P, D], bf16, tag="dump")
                    dump2 = dumpp.tile([P, D], bf16, tag="dump2")
                    sq = small.tile([P, J], f32, tag="sq")
                    for j in range(J):
                        if j < 2:
                            # DVE one-pass square+accumulate (bf16 in, fp32 accum)
                            nc.vector.scalar_tensor_tensor(
                                out=dump[:],
                                in0=xb[:, j, :],
                                scalar=1.0,
                                in1=xb[:, j, :],
                                op0=OP.mult,
                                op1=OP.mult,
                                accum_out=sq[:, j : j + 1],
                            )
                        else:
                            # ACT square+accumulate (parallel engine)
                            nc.scalar.activation(
                                out=dump2[:],
                                in_=xb[:, j, :],
                                func=AF.Square,
                                accum_out=sq[:, j : j + 1],
                            )
                    isq = small.tile([P, J], f32, tag="isq")
                    nc.vector.reciprocal(out=isq[:], in_=sq[:])
                    inv = small.tile([P, J], f32, tag="inv")
                    nc.scalar.activation(out=inv[:], in_=isq[:], func=AF.Sqrt)
                    # per-block weight tile: (ent, nz/||x||) interleaved, bf16
                    w_t = wp.tile([P, J, 2], bf16)
                    nc.scalar.copy(out=w_t[:, :, 0], in_=aux_sb[:, i, :, 0])
                    nc.vector.tensor_tensor(
                        out=w_t[:, :, 1],
                        in0=aux_sb[:, i, :, 1],
                        in1=inv[:],
                        op=OP.mult,
                    )
                    for j in range(J):
                        w = w_t[:, j, :]            # [128, 2]
                        first = i == 0 and j == 0
                        last = i == N_BLK - 1 and j == J - 1
                        nc.tensor.matmul(
                            p512[:], w, xb[:, j, 0:512], start=first, stop=last
                        )
                        nc.tensor.matmul(
                            p256[:], w, xb[:, j, 512:768], start=first, stop=last
                        )

            out_sb = single.tile([2, D], f32)
            nc.vector.tensor_copy(out=out_sb[:, 0:512], in_=p512[:])
            nc.vector.tensor_copy(out=out_sb[:, 512:768], in_=p256[:])
            nc.sync.dma_start(out=out_h[:], in_=out_sb[:])


def _build():
    """Manual module build, used for CoreSim validation only."""
    import concourse.bacc as bacc
    from concourse import mybir

    f32 = mybir.dt.float32
    nc = bacc.Bacc("TRN2", target_bir_lowering=False, debug=False)
    x_dram = nc.dram_tensor("x", [N_BLK, P, J, D], f32, kind="ExternalInput")
    aux_dram = nc.dram_tensor("aux", [P, N_BLK, J, 2], f32, kind="ExternalInput")
    out_dram = nc.dram_tensor("out", [2, D], f32, kind="ExternalOutput")
    _tile_program(nc, x_dram, aux_dram, out_dram)
    nc.finalize()
    return nc


def _get_nc():
    if "nc" not in _CACHE:
        _CACHE["nc"] = _build()
    return _CACHE["nc"]


def _get_sharded_fn():
    """bass_jit kernel shard_mapped over the 8 cores (the proven exec path)."""
    if "fn" in _CACHE:
        return _CACHE["fn"]
    import jax
    from jax.sharding import Mesh, PartitionSpec
    from concourse.bass2jax import bass_jit, bass_shard_map
    from concourse import mybir

    f32 = mybir.dt.float32

    @bass_jit
    def body(nc, x, aux):
        out = nc.dram_tensor("out", [2, D], f32, kind="ExternalOutput")
        _tile_program(nc, x, aux, out)
        return out

    devices = jax.devices()[:N_CORES]
    mesh = Mesh(np.asarray(devices), ("core",))
    fn = bass_shard_map(
        body,
        mesh=mesh,
        in_specs=(PartitionSpec("core"), PartitionSpec("core")),
        out_specs=PartitionSpec("core"),
    )
    _CACHE["fn"] = fn
    return fn


def _make_in_maps(logits, labels, entity_id):
    logits = np.asarray(logits).astype(np.float32, copy=False).reshape(B, S, D)
    labels = np.asarray(labels).reshape(B, S).astype(np.int64, copy=False)
    eid = int(np.asarray(entity_id))

    pos_ok = np.arange(S)[None, :] != 0
    ent = ((labels == eid) & pos_ok).astype(np.float32).reshape(-1)
    nz = (labels != 0).astype(np.float32).reshape(-1)

    in_maps = []
    for c in range(N_CORES):
        shard = logits[c * B_PER_CORE : (c + 1) * B_PER_CORE]
        x = np.ascontiguousarray(shard.reshape(N_BLK, P, J, D))
        sl = slice(c * TOK_PER_CORE, (c + 1) * TOK_PER_CORE)
        ent_c = ent[sl].reshape(N_BLK, P, J)
        nz_c = nz[sl].reshape(N_BLK, P, J)
        aux = np.ascontiguousarray(
            np.stack([ent_c, nz_c], axis=-1).transpose(1, 0, 2, 3)
        )  # [P, N_BLK, J, 2]
        in_maps.append({"x": x, "aux": aux})

    c1 = max(float(ent.sum()), 1.0)
    c2 = max(float(nz.sum()), 1.0)
    return in_maps, c1, c2


def _combine(partials, c1, c2):
    """partials: list of [2, D] float arrays (one per core)."""
    acc = np.zeros((2, D), dtype=np.float64)
    for p in partials:
        acc += np.asarray(p, dtype=np.float64)
    v1, v2 = acc[0], acc[1]
    proto = v1 / c1
    pn = float(np.sqrt((proto * proto).sum()))
    if pn < 1e-30:
        return np.float32(0.0)
    loss = float(v2 @ proto) / (pn * c2)
    return np.float32(loss)


def _run_hw(in_maps):
    """Run the 8-core shard_map; returns list of [2, D] partials."""
    fn = _get_sharded_fn()
    x_g = np.concatenate([m["x"] for m in in_maps], axis=0)
    aux_g = np.concatenate([m["aux"] for m in in_maps], axis=0)
    out = np.asarray(fn(x_g, aux_g))  # [2 * N_CORES, D]
    return [out[2 * c : 2 * c + 2] for c in range(N_CORES)]


def kernel(logits, labels, entity_id):
    in_maps, c1, c2 = _make_in_maps(logits, labels, entity_id)
    partials = _run_hw(in_maps)
    return _combine(partials, c1, c2)

